# revision 22
# baseline (speedup 1.0000x reference)
"""Trainium2 Bass kernel for nn_BoostEnhancedAttention.

Reference computation:
    v   = (values @ W_v.T + b_v)                      # [B, NK, H*D_V]
    att = softmax(att3 ⊗ att12 interleaved, axis=k)   # [B, H, NQ, NK]
    out = (att @ v_per_head) @ W_o.T + b_o            # [B, NQ, D_MODEL]

Restructuring used here (exact algebra, verified vs reference):
  - Scores factor as s[b,h,q,k] = att3[b,h,q,c(k)] * att12[b,h,...f(k)], so
    exp(s) is computed by the ACT engine directly with the multiply folded
    into the activation's per-partition `scale` operand. No separate score
    build pass.
  - Since softmax rows sum to 1 and both projections are linear, fold
    W_v/W_o into per-head M_h = W_o[:,h] @ W_v[h,:] and apply AFTER
    attention:  out[b] = sum_h (att_h @ values[b]) @ M_h.T + b_eff.
    This lets the attention matmul consume `values` in natural [k, d]
    layout (k on partitions) — no transpose of the big tensor anywhere.
  - Softmax normalization deferred: G~ = E @ values accumulated
    unnormalized in PSUM; Z = column sums of E obtained with a ones-matmul
    (output replicated across all 128 partitions so the normalizing
    multiply needs no partition broadcast).

Sharding: data-parallel over batch, B=32 over 8 cores -> 4 batches/core.
No collectives needed; outputs concatenated on host.
"""

import numpy as np
import ml_dtypes

B, CH, CW, H, FH, FW = 32, 16, 16, 8, 4, 4
NQ = 64
NCELL = CH * CW          # 256 coarse cells (c)
F = FH * FW              # 16 fine positions per cell
NK = NCELL * F           # 4096
D_IN, D_V, D_MODEL = 512, 64, 512
N_CORES = 8
B_LOC = B // N_CORES     # 4
N_KT = 32                # k-tiles of 128: kt = half*16 + f, partition = c_loc
N_DT = 4                 # d_in tiles of 128
HQ = H * NQ              # 512

BF16 = ml_dtypes.bfloat16


def _k_perm():
    """perm[k'] -> original k, where k' = (half*16+f)*128 + c_loc.

    Original key order is (ch, fh, cw, fw):  k = ch*256 + fh*64 + cw*4 + fw.
    New order groups a k-tile as (fixed f=(fh,fw), c = half*128 + c_loc).
    """
    perm = np.zeros(NK, np.int64)
    c = np.arange(NCELL)
    ch_i, cw_i = c // CW, c % CW
    for half in range(2):
        for f in range(F):
            kt = half * F + f
            fh, fw = f // FW, f % FW
            cc = half * 128 + np.arange(128)
            perm[kt * 128:(kt + 1) * 128] = (
                ch_i[cc] * (FH * CW * FW) + fh * (CW * FW) + cw_i[cc] * FW + fw
            )
    return perm


_PERM = _k_perm()
_NC_CACHE = {}


def _build_nc():
    from contextlib import ExitStack

    import concourse.bass as bass
    import concourse.tile as tile
    from concourse import bacc, mybir

    f32 = mybir.dt.float32
    bf16 = mybir.dt.bfloat16

    nc = bacc.Bacc("TRN2", target_bir_lowering=False, debug=False,
                   num_devices=N_CORES)

    values_r = nc.dram_tensor("values_r", [B_LOC, NK, D_IN], bf16,
                              kind="ExternalInput")
    att3_t = nc.dram_tensor("att3_t", [B_LOC, NCELL, HQ], bf16,
                            kind="ExternalInput")
    att12_pair = nc.dram_tensor("att12_pair", [B_LOC, NCELL, F * H * 2], bf16,
                                kind="ExternalInput")
    m_all = nc.dram_tensor("m_all", [128, N_DT * H * D_MODEL], bf16,
                           kind="ExternalInput")
    beff = nc.dram_tensor("beff", [1, D_MODEL], bf16, kind="ExternalInput")
    out = nc.dram_tensor("out", [B_LOC * NQ, D_MODEL], f32,
                         kind="ExternalOutput")

    with tile.TileContext(nc) as tc, ExitStack() as ctx:
        const_pool = ctx.enter_context(tc.tile_pool(name="const", bufs=1))
        a3_pool = ctx.enter_context(tc.tile_pool(name="a3", bufs=2))
        a12r_pool = ctx.enter_context(tc.tile_pool(name="a12r", bufs=2))
        vt_pool = ctx.enter_context(tc.tile_pool(name="vt", bufs=16))
        sc_pool = ctx.enter_context(tc.tile_pool(name="sc", bufs=3))
        et_pool = ctx.enter_context(tc.tile_pool(name="et", bufs=3))
        esum_pool = ctx.enter_context(tc.tile_pool(name="esum", bufs=2))
        zb_pool = ctx.enter_context(tc.tile_pool(name="zb", bufs=2))
        g_pool = ctx.enter_context(tc.tile_pool(name="gps", bufs=1, space="PSUM"))
        z_pool = ctx.enter_context(tc.tile_pool(name="zps", bufs=1, space="PSUM"))
        o_pool = ctx.enter_context(tc.tile_pool(name="ops", bufs=1, space="PSUM"))
        o_sb_pool = ctx.enter_context(tc.tile_pool(name="osb", bufs=2))

        ones_sb = const_pool.tile([128, 128], bf16)
        nc.vector.memset(ones_sb[:], 1.0)
        warm_sb = const_pool.tile([128, D_MODEL], bf16, name="warm_sb")
        nc.vector.memset(warm_sb[:], 1.0)
        warm = o_pool.tile([128, D_MODEL], f32, tag="o", name="warm")
        for wi in range(12):
            nc.tensor.matmul(warm[:], ones_sb[:], warm_sb[:],
                             start=True, stop=True)
        beff_sb = const_pool.tile([1, D_MODEL], bf16)
        nc.sync.dma_start(beff_sb[:], beff.ap())
        # g_all[d_loc, (dt, h, b, q)] : normalized attention output, bf16
        g_all = const_pool.tile([128, N_DT * H * B_LOC * NQ], bf16)

        Q2 = NQ // 2

        def emit_group(b, half, gi, FQ, f0, a3_t, a12r_t):
            """One score group: broadcast multiply + exp for FQ f-positions."""
            a3b = a3_t[half][:]
            in0 = bass.AP(a3b.tensor, a3b.offset,
                          [a3b.ap[0], [0, FQ], [NQ, H], [2, Q2], [1, 2]])
            sc = sc_pool.tile([128, 4 * HQ], bf16, tag="sc",
                              name=f"sc_{b}_{half}_{gi}")
            scb = sc[:]
            out_ap = bass.AP(scb.tensor, scb.offset,
                             [scb.ap[0], [HQ, FQ], [NQ, H], [2, Q2], [1, 2]])
            a12b = a12r_t[half][:]
            in1 = bass.AP(a12b.tensor, a12b.offset + f0 * H * 2,
                          [a12b.ap[0], [H * 2, FQ], [2, H], [0, Q2], [1, 2]])
            nc.vector.tensor_mul(out_ap, in0, in1)
            et = et_pool.tile([128, 4 * HQ], bf16, tag="et",
                              name=f"et_{b}_{half}_{gi}")
            nc.scalar.activation(et[:, :FQ * HQ], sc[:, :FQ * HQ],
                                 mybir.ActivationFunctionType.Exp)
            return et

        def prologue(b):
            """Input DMAs + first score group for batch b — emitted ahead of
            the previous batch's epilogue so the DVE/ACT pipeline stays
            primed across the batch transition."""
            a3_t = [a3_pool.tile([128, HQ], bf16, tag=f"a3_{hf}",
                                 name=f"a3_{b}_{hf}") for hf in range(2)]
            for hf in range(2):
                nc.sync.dma_start(a3_t[hf][:],
                                  att3_t.ap()[b, hf * 128:(hf + 1) * 128, :])
            a12r_t = []
            for hf in range(2):
                a12r = a12r_pool.tile([128, F * H * 2], bf16, tag=f"a12r_{hf}",
                                      name=f"a12r_{b}_{hf}")
                nc.sync.dma_start(a12r[:],
                                  att12_pair.ap()[b, hf * 128:(hf + 1) * 128, :])
                a12r_t.append(a12r)
            groups = [1, 1, 2, 4, 4, 4] if b == 0 else [4, 4, 4, 4]
            et0 = emit_group(b, 0, 0, groups[0], 0, a3_t, a12r_t)
            return a3_t, a12r_t, groups, et0

        pro = prologue(0)
        for b in range(B_LOC):
            a3_t, a12r_t, groups0, et0 = pro
            gps = [g_pool.tile([128, HQ], f32, tag=f"g{dt}", name=f"g_{b}_{dt}",
                               bufs=(2 if dt < 2 else 1))
                   for dt in range(N_DT)]
            esum = esum_pool.tile([128, HQ], bf16)

            for half in range(2):
                groups = groups0 if half == 0 else [4, 4, 4, 4]
                f0 = 0
                for gi, FQ in enumerate(groups):
                    if half == 0 and gi == 0:
                        et = et0
                    else:
                        et = emit_group(b, half, gi, FQ, f0, a3_t, a12r_t)
                    for j in range(FQ):
                        kt = half * F + f0 + j
                        vt = vt_pool.tile([128, D_IN], bf16, tag="vt",
                                          name=f"vt_{b}_{kt}")
                        nc.sync.dma_start(
                            vt[:], values_r.ap()[b, kt * 128:(kt + 1) * 128, :])
                        ets = et[:, j * HQ:(j + 1) * HQ]
                        DEFER = 3
                        if kt < DEFER:
                            if kt == 0:
                                deferred = []
                            for dt in range(2):
                                nc.tensor.matmul(gps[dt][:],
                                                 vt[:, dt * 128:(dt + 1) * 128],
                                                 ets, start=(kt == 0),
                                                 stop=False)
                            deferred.append((vt, ets, kt == 0))
                            if kt == DEFER - 1:
                                for dvt, dets, dstart in deferred:
                                    for dt in range(2, N_DT):
                                        nc.tensor.matmul(
                                            gps[dt][:],
                                            dvt[:, dt * 128:(dt + 1) * 128],
                                            dets, start=dstart, stop=False)
                        else:
                            for dt in range(N_DT):
                                nc.tensor.matmul(gps[dt][:],
                                                 vt[:, dt * 128:(dt + 1) * 128],
                                                 ets,
                                                 start=False,
                                                 stop=(kt == N_KT - 1))
                        if kt == 0:
                            nc.vector.tensor_copy(esum[:], ets)
                        else:
                            nc.vector.tensor_add(esum[:], esum[:], ets)
                    f0 += FQ

            if b + 1 < B_LOC:
                pro = prologue(b + 1)

            if b == 0:
                # deferred so startup DMA bandwidth goes to the first tiles
                m_sb = const_pool.tile([128, N_DT * H * D_MODEL], bf16,
                                       name="m_sb")
                mq = N_DT * H * D_MODEL // 4
                for mi in range(4):
                    nc.sync.dma_start(m_sb[:, mi * mq:(mi + 1) * mq],
                                      m_all.ap()[:, mi * mq:(mi + 1) * mq])

            # Z replicated on all partitions via ones-matmul, then 1/Z
            zps = z_pool.tile([128, HQ], f32, tag="z")
            nc.tensor.matmul(zps[:], ones_sb[:], esum[:], start=True, stop=True)
            zb = zb_pool.tile([128, HQ], f32)
            nc.vector.reciprocal_approx_fast(zb[:], zps[:])

            ga_v = g_all[:].rearrange("p (dt h bb q) -> p dt h bb q",
                                      dt=N_DT, h=H, bb=B_LOC)
            for dt in range(N_DT):
                nc.vector.tensor_mul(
                    ga_v[:, dt, :, b, :],
                    gps[dt][:].rearrange("p (h q) -> p h q", h=H),
                    zb[:].rearrange("p (h q) -> p h q", h=H),
                )

            # Output projection for each completed pair of batches
            if b % 2 == 1:
                bq = b // 2
                ops = o_pool.tile([128, D_MODEL], f32, tag="o")
                i = 0
                for dt in range(N_DT):
                    for h in range(H):
                        col = dt * (H * B_LOC * NQ) + h * (B_LOC * NQ) + bq * 128
                        nc.tensor.matmul(
                            ops[:],
                            g_all[:, col:col + 128],
                            m_sb[:, (dt * H + h) * D_MODEL:
                                 (dt * H + h + 1) * D_MODEL],
                            start=(i == 0), stop=False)
                        i += 1
                # bias via K=1 matmul (broadcasts b_eff to all partitions)
                nc.tensor.matmul(ops[:], ones_sb[0:1, :], beff_sb[:],
                                 start=False, stop=True)
                out_sb = o_sb_pool.tile([128, D_MODEL], f32, tag="osb",
                                        name=f"osb_{bq}")
                nc.vector.tensor_copy(out_sb[:], ops[:])
                nc.sync.dma_start(out.ap()[bq * 128:(bq + 1) * 128, :],
                                  out_sb[:])

    nc.compile()
    return nc


def _get_nc():
    if "nc" not in _NC_CACHE:
        _NC_CACHE["nc"] = _build_nc()
    return _NC_CACHE["nc"]


def _host_prep(att12, att3, values, W_v, b_v, W_o, b_o):
    att12 = np.asarray(att12, np.float32)
    att3 = np.asarray(att3, np.float32)
    values = np.asarray(values, np.float32)
    W_v = np.asarray(W_v, np.float32)
    b_v = np.asarray(b_v, np.float32)
    W_o = np.asarray(W_o, np.float32)
    b_o = np.asarray(b_o, np.float32)

    values_r = np.ascontiguousarray(values[:, _PERM, :]).astype(BF16)
    att3_t = np.ascontiguousarray(
        att3.transpose(0, 3, 1, 2).reshape(B, NCELL, HQ)).astype(BF16)
    att12_r = np.ascontiguousarray(
        att12.transpose(0, 1, 2, 4, 5, 3).reshape(B, NCELL, F * H)).astype(BF16)
    att12_pair = np.ascontiguousarray(np.broadcast_to(
        att12_r[:, :, :, None], (B, NCELL, F * H, 2)).reshape(
        B, NCELL, F * H * 2))

    # Per-head folded projection M_h = W_o_h @ W_v_h  [D_MODEL, D_IN]
    Wv3 = W_v.reshape(H, D_V, D_IN)
    Wo3 = W_o.reshape(D_MODEL, H, D_V)
    M = np.einsum("dhv,hvi->hdi", Wo3, Wv3)          # [H, DM, DIN]
    # m_all[d_loc, (dt, h, dm)] = M[h].T[dt*128+d_loc, dm]
    Mt = M.transpose(0, 2, 1)                        # [H, DIN, DM]
    m_all = np.ascontiguousarray(
        Mt.reshape(H, N_DT, 128, D_MODEL).transpose(2, 1, 0, 3)
        .reshape(128, N_DT * H * D_MODEL)).astype(BF16)

    b_eff = b_o + np.einsum("dhv,hv->d", Wo3, b_v.reshape(H, D_V))
    beff = b_eff.reshape(1, D_MODEL).astype(BF16)
    return values_r, att3_t, att12_pair, m_all, beff


def kernel(att12, att3, values, W_v, b_v, W_o, b_o):
    from concourse.bass_utils import run_bass_kernel_spmd

    values_r, att3_t, att12_pair, m_all, beff = _host_prep(
        att12, att3, values, W_v, b_v, W_o, b_o)

    in_maps = []
    for core in range(N_CORES):
        s = slice(core * B_LOC, (core + 1) * B_LOC)
        in_maps.append({
            "values_r": np.ascontiguousarray(values_r[s]),
            "att3_t": np.ascontiguousarray(att3_t[s]),
            "att12_pair": np.ascontiguousarray(att12_pair[s]),
            "m_all": m_all,
            "beff": beff,
        })

    nc = _get_nc()
    res = run_bass_kernel_spmd(nc, in_maps, core_ids=list(range(N_CORES)))
    out = np.concatenate(
        [res.results[i]["out"].reshape(B_LOC, NQ, D_MODEL)
         for i in range(N_CORES)], axis=0)
    return out.astype(np.float32)


# revision 23
# speedup vs baseline: 1.0213x; 1.0213x over previous
"""Trainium2 Bass kernel for nn_BoostEnhancedAttention.

Reference computation:
    v   = (values @ W_v.T + b_v)                      # [B, NK, H*D_V]
    att = softmax(att3 ⊗ att12 interleaved, axis=k)   # [B, H, NQ, NK]
    out = (att @ v_per_head) @ W_o.T + b_o            # [B, NQ, D_MODEL]

Restructuring used here (exact algebra, verified vs reference):
  - Scores factor as s[b,h,q,k] = att3[b,h,q,c(k)] * att12[b,h,...f(k)], so
    exp(s) is computed by the ACT engine directly with the multiply folded
    into the activation's per-partition `scale` operand. No separate score
    build pass.
  - Since softmax rows sum to 1 and both projections are linear, fold
    W_v/W_o into per-head M_h = W_o[:,h] @ W_v[h,:] and apply AFTER
    attention:  out[b] = sum_h (att_h @ values[b]) @ M_h.T + b_eff.
    This lets the attention matmul consume `values` in natural [k, d]
    layout (k on partitions) — no transpose of the big tensor anywhere.
  - Softmax normalization deferred: G~ = E @ values accumulated
    unnormalized in PSUM; Z = column sums of E obtained with a ones-matmul
    (output replicated across all 128 partitions so the normalizing
    multiply needs no partition broadcast).

Sharding: data-parallel over batch, B=32 over 8 cores -> 4 batches/core.
No collectives needed; outputs concatenated on host.
"""

import numpy as np
import ml_dtypes

B, CH, CW, H, FH, FW = 32, 16, 16, 8, 4, 4
NQ = 64
NCELL = CH * CW          # 256 coarse cells (c)
F = FH * FW              # 16 fine positions per cell
NK = NCELL * F           # 4096
D_IN, D_V, D_MODEL = 512, 64, 512
N_CORES = 8
B_LOC = B // N_CORES     # 4
N_KT = 32                # k-tiles of 128: kt = half*16 + f, partition = c_loc
N_DT = 4                 # d_in tiles of 128
HQ = H * NQ              # 512

BF16 = ml_dtypes.bfloat16


def _k_perm():
    """perm[k'] -> original k, where k' = (half*16+f)*128 + c_loc.

    Original key order is (ch, fh, cw, fw):  k = ch*256 + fh*64 + cw*4 + fw.
    New order groups a k-tile as (fixed f=(fh,fw), c = half*128 + c_loc).
    """
    perm = np.zeros(NK, np.int64)
    c = np.arange(NCELL)
    ch_i, cw_i = c // CW, c % CW
    for half in range(2):
        for f in range(F):
            kt = half * F + f
            fh, fw = f // FW, f % FW
            cc = half * 128 + np.arange(128)
            perm[kt * 128:(kt + 1) * 128] = (
                ch_i[cc] * (FH * CW * FW) + fh * (CW * FW) + cw_i[cc] * FW + fw
            )
    return perm


_PERM = _k_perm()
_NC_CACHE = {}


def _build_nc():
    from contextlib import ExitStack

    import concourse.bass as bass
    import concourse.tile as tile
    from concourse import bacc, mybir

    f32 = mybir.dt.float32
    bf16 = mybir.dt.bfloat16

    nc = bacc.Bacc("TRN2", target_bir_lowering=False, debug=False,
                   num_devices=N_CORES)

    values_r = nc.dram_tensor("values_r", [B_LOC, NK, D_IN], bf16,
                              kind="ExternalInput")
    att3_t = nc.dram_tensor("att3_t", [B_LOC, NCELL, HQ], bf16,
                            kind="ExternalInput")
    att12_pair = nc.dram_tensor("att12_pair", [B_LOC, NCELL, F * H * 2], bf16,
                                kind="ExternalInput")
    m_all = nc.dram_tensor("m_all", [128, N_DT * H * D_MODEL], bf16,
                           kind="ExternalInput")
    beff = nc.dram_tensor("beff", [1, D_MODEL], bf16, kind="ExternalInput")
    out = nc.dram_tensor("out", [B_LOC * NQ, D_MODEL], f32,
                         kind="ExternalOutput")

    with tile.TileContext(nc) as tc, ExitStack() as ctx:
        const_pool = ctx.enter_context(tc.tile_pool(name="const", bufs=1))
        a3_pool = ctx.enter_context(tc.tile_pool(name="a3", bufs=2))
        a12r_pool = ctx.enter_context(tc.tile_pool(name="a12r", bufs=2))
        vt_pool = ctx.enter_context(tc.tile_pool(name="vt", bufs=16))
        sc_pool = ctx.enter_context(tc.tile_pool(name="sc", bufs=3))
        et_pool = ctx.enter_context(tc.tile_pool(name="et", bufs=3))
        esum_pool = ctx.enter_context(tc.tile_pool(name="esum", bufs=2))
        zb_pool = ctx.enter_context(tc.tile_pool(name="zb", bufs=2))
        g_pool = ctx.enter_context(tc.tile_pool(name="gps", bufs=1, space="PSUM"))
        z_pool = ctx.enter_context(tc.tile_pool(name="zps", bufs=1, space="PSUM"))
        o_pool = ctx.enter_context(tc.tile_pool(name="ops", bufs=1, space="PSUM"))
        o_sb_pool = ctx.enter_context(tc.tile_pool(name="osb", bufs=2))

        ones_sb = const_pool.tile([128, 128], bf16)
        nc.vector.memset(ones_sb[:], 1.0)
        warm_sb = const_pool.tile([128, D_MODEL], bf16, name="warm_sb")
        nc.vector.memset(warm_sb[:], 1.0)
        warm = o_pool.tile([128, D_MODEL], f32, tag="o", name="warm")
        for wi in range(12):
            nc.tensor.matmul(warm[:], ones_sb[:], warm_sb[:],
                             start=True, stop=True)
        beff_sb = const_pool.tile([1, D_MODEL], bf16)
        nc.sync.dma_start(beff_sb[:], beff.ap())
        # g_all[d_loc, (dt, h, b, q)] : normalized attention output, bf16
        g_all = const_pool.tile([128, N_DT * H * B_LOC * NQ], bf16)

        Q2 = NQ // 2

        def emit_group(b, half, gi, FQ, f0, a3_t, a12r_t):
            """One score group: broadcast multiply + exp for FQ f-positions."""
            a3b = a3_t[half][:]
            in0 = bass.AP(a3b.tensor, a3b.offset,
                          [a3b.ap[0], [0, FQ], [NQ, H], [2, Q2], [1, 2]])
            sc = sc_pool.tile([128, 4 * HQ], bf16, tag="sc",
                              name=f"sc_{b}_{half}_{gi}")
            scb = sc[:]
            out_ap = bass.AP(scb.tensor, scb.offset,
                             [scb.ap[0], [HQ, FQ], [NQ, H], [2, Q2], [1, 2]])
            a12b = a12r_t[half][:]
            in1 = bass.AP(a12b.tensor, a12b.offset + f0 * H * 2,
                          [a12b.ap[0], [H * 2, FQ], [2, H], [0, Q2], [1, 2]])
            nc.vector.tensor_mul(out_ap, in0, in1)
            et = et_pool.tile([128, 4 * HQ], bf16, tag="et",
                              name=f"et_{b}_{half}_{gi}")
            nc.scalar.activation(et[:, :FQ * HQ], sc[:, :FQ * HQ],
                                 mybir.ActivationFunctionType.Exp)
            return et

        def prologue(b):
            """Input DMAs + first score group for batch b — emitted ahead of
            the previous batch's epilogue so the DVE/ACT pipeline stays
            primed across the batch transition."""
            a3_t = [a3_pool.tile([128, HQ], bf16, tag=f"a3_{hf}",
                                 name=f"a3_{b}_{hf}") for hf in range(2)]
            for hf in range(2):
                nc.sync.dma_start(a3_t[hf][:],
                                  att3_t.ap()[b, hf * 128:(hf + 1) * 128, :])
            a12r_t = []
            for hf in range(2):
                a12r = a12r_pool.tile([128, F * H * 2], bf16, tag=f"a12r_{hf}",
                                      name=f"a12r_{b}_{hf}")
                nc.sync.dma_start(a12r[:],
                                  att12_pair.ap()[b, hf * 128:(hf + 1) * 128, :])
                a12r_t.append(a12r)
            groups = [1, 1, 2, 4, 4, 4] if b == 0 else [4, 4, 4, 4]
            et0 = emit_group(b, 0, 0, groups[0], 0, a3_t, a12r_t)
            return a3_t, a12r_t, groups, et0

        pro = prologue(0)
        for b in range(B_LOC):
            a3_t, a12r_t, groups0, et0 = pro
            gps = [g_pool.tile([128, HQ], f32, tag=f"g{dt}", name=f"g_{b}_{dt}",
                               bufs=(2 if dt < 2 else 1))
                   for dt in range(N_DT)]
            esum = esum_pool.tile([128, HQ], bf16)

            for half in range(2):
                groups = groups0 if half == 0 else [4, 4, 4, 4]
                f0 = 0
                for gi, FQ in enumerate(groups):
                    if half == 0 and gi == 0:
                        et = et0
                    else:
                        et = emit_group(b, half, gi, FQ, f0, a3_t, a12r_t)
                    for j in range(FQ):
                        kt = half * F + f0 + j
                        vt = vt_pool.tile([128, D_IN], bf16, tag="vt",
                                          name=f"vt_{b}_{kt}")
                        nc.sync.dma_start(
                            vt[:], values_r.ap()[b, kt * 128:(kt + 1) * 128, :])
                        ets = et[:, j * HQ:(j + 1) * HQ]
                        DEFER = 3
                        if kt < DEFER:
                            if kt == 0:
                                deferred = []
                            for dt in range(2):
                                nc.tensor.matmul(gps[dt][:],
                                                 vt[:, dt * 128:(dt + 1) * 128],
                                                 ets, start=(kt == 0),
                                                 stop=False)
                            deferred.append((vt, ets, kt == 0))
                            if kt == DEFER - 1:
                                for dvt, dets, dstart in deferred:
                                    for dt in range(2, N_DT):
                                        nc.tensor.matmul(
                                            gps[dt][:],
                                            dvt[:, dt * 128:(dt + 1) * 128],
                                            dets, start=dstart, stop=False)
                        else:
                            for dt in range(N_DT):
                                nc.tensor.matmul(gps[dt][:],
                                                 vt[:, dt * 128:(dt + 1) * 128],
                                                 ets,
                                                 start=False,
                                                 stop=(kt == N_KT - 1))
                        if kt == 0:
                            nc.vector.tensor_copy(esum[:], ets)
                        else:
                            nc.vector.tensor_add(esum[:], esum[:], ets)
                    f0 += FQ
                if b == 1 and half == 0:
                    # weights for the output projection, emitted mid-stream so
                    # the transfer never contends with critical prefetches
                    m_sb = const_pool.tile([128, N_DT * H * D_MODEL], bf16,
                                           name="m_sb")
                    mq = N_DT * H * D_MODEL // 4
                    for mi in range(4):
                        nc.sync.dma_start(m_sb[:, mi * mq:(mi + 1) * mq],
                                          m_all.ap()[:, mi * mq:(mi + 1) * mq])

            if b + 1 < B_LOC:
                pro = prologue(b + 1)

            # Z replicated on all partitions via ones-matmul, then 1/Z
            zps = z_pool.tile([128, HQ], f32, tag="z")
            nc.tensor.matmul(zps[:], ones_sb[:], esum[:], start=True, stop=True)
            zb = zb_pool.tile([128, HQ], f32)
            nc.vector.reciprocal_approx_fast(zb[:], zps[:])

            ga_v = g_all[:].rearrange("p (dt h bb q) -> p dt h bb q",
                                      dt=N_DT, h=H, bb=B_LOC)
            for dt in range(N_DT):
                nc.vector.tensor_mul(
                    ga_v[:, dt, :, b, :],
                    gps[dt][:].rearrange("p (h q) -> p h q", h=H),
                    zb[:].rearrange("p (h q) -> p h q", h=H),
                )

            # Output projection for each completed pair of batches
            if b % 2 == 1:
                bq = b // 2
                ops = o_pool.tile([128, D_MODEL], f32, tag="o")
                i = 0
                for dt in range(N_DT):
                    for h in range(H):
                        col = dt * (H * B_LOC * NQ) + h * (B_LOC * NQ) + bq * 128
                        nc.tensor.matmul(
                            ops[:],
                            g_all[:, col:col + 128],
                            m_sb[:, (dt * H + h) * D_MODEL:
                                 (dt * H + h + 1) * D_MODEL],
                            start=(i == 0), stop=False)
                        i += 1
                # bias via K=1 matmul (broadcasts b_eff to all partitions)
                nc.tensor.matmul(ops[:], ones_sb[0:1, :], beff_sb[:],
                                 start=False, stop=True)
                out_sb = o_sb_pool.tile([128, D_MODEL], f32, tag="osb",
                                        name=f"osb_{bq}")
                nc.vector.tensor_copy(out_sb[:], ops[:])
                nc.sync.dma_start(out.ap()[bq * 128:(bq + 1) * 128, :],
                                  out_sb[:])

    nc.compile()
    return nc


def _get_nc():
    if "nc" not in _NC_CACHE:
        _NC_CACHE["nc"] = _build_nc()
    return _NC_CACHE["nc"]


def _host_prep(att12, att3, values, W_v, b_v, W_o, b_o):
    att12 = np.asarray(att12, np.float32)
    att3 = np.asarray(att3, np.float32)
    values = np.asarray(values, np.float32)
    W_v = np.asarray(W_v, np.float32)
    b_v = np.asarray(b_v, np.float32)
    W_o = np.asarray(W_o, np.float32)
    b_o = np.asarray(b_o, np.float32)

    values_r = np.ascontiguousarray(values[:, _PERM, :]).astype(BF16)
    att3_t = np.ascontiguousarray(
        att3.transpose(0, 3, 1, 2).reshape(B, NCELL, HQ)).astype(BF16)
    att12_r = np.ascontiguousarray(
        att12.transpose(0, 1, 2, 4, 5, 3).reshape(B, NCELL, F * H)).astype(BF16)
    att12_pair = np.ascontiguousarray(np.broadcast_to(
        att12_r[:, :, :, None], (B, NCELL, F * H, 2)).reshape(
        B, NCELL, F * H * 2))

    # Per-head folded projection M_h = W_o_h @ W_v_h  [D_MODEL, D_IN]
    Wv3 = W_v.reshape(H, D_V, D_IN)
    Wo3 = W_o.reshape(D_MODEL, H, D_V)
    M = np.einsum("dhv,hvi->hdi", Wo3, Wv3)          # [H, DM, DIN]
    # m_all[d_loc, (dt, h, dm)] = M[h].T[dt*128+d_loc, dm]
    Mt = M.transpose(0, 2, 1)                        # [H, DIN, DM]
    m_all = np.ascontiguousarray(
        Mt.reshape(H, N_DT, 128, D_MODEL).transpose(2, 1, 0, 3)
        .reshape(128, N_DT * H * D_MODEL)).astype(BF16)

    b_eff = b_o + np.einsum("dhv,hv->d", Wo3, b_v.reshape(H, D_V))
    beff = b_eff.reshape(1, D_MODEL).astype(BF16)
    return values_r, att3_t, att12_pair, m_all, beff


def kernel(att12, att3, values, W_v, b_v, W_o, b_o):
    from concourse.bass_utils import run_bass_kernel_spmd

    values_r, att3_t, att12_pair, m_all, beff = _host_prep(
        att12, att3, values, W_v, b_v, W_o, b_o)

    in_maps = []
    for core in range(N_CORES):
        s = slice(core * B_LOC, (core + 1) * B_LOC)
        in_maps.append({
            "values_r": np.ascontiguousarray(values_r[s]),
            "att3_t": np.ascontiguousarray(att3_t[s]),
            "att12_pair": np.ascontiguousarray(att12_pair[s]),
            "m_all": m_all,
            "beff": beff,
        })

    nc = _get_nc()
    res = run_bass_kernel_spmd(nc, in_maps, core_ids=list(range(N_CORES)))
    out = np.concatenate(
        [res.results[i]["out"].reshape(B_LOC, NQ, D_MODEL)
         for i in range(N_CORES)], axis=0)
    return out.astype(np.float32)


# revision 24
# speedup vs baseline: 1.0366x; 1.0150x over previous
"""Trainium2 Bass kernel for nn_BoostEnhancedAttention.

Reference computation:
    v   = (values @ W_v.T + b_v)                      # [B, NK, H*D_V]
    att = softmax(att3 ⊗ att12 interleaved, axis=k)   # [B, H, NQ, NK]
    out = (att @ v_per_head) @ W_o.T + b_o            # [B, NQ, D_MODEL]

Restructuring used here (exact algebra, verified vs reference):
  - Scores factor as s[b,h,q,k] = att3[b,h,q,c(k)] * att12[b,h,...f(k)], so
    exp(s) is computed by the ACT engine directly with the multiply folded
    into the activation's per-partition `scale` operand. No separate score
    build pass.
  - Since softmax rows sum to 1 and both projections are linear, fold
    W_v/W_o into per-head M_h = W_o[:,h] @ W_v[h,:] and apply AFTER
    attention:  out[b] = sum_h (att_h @ values[b]) @ M_h.T + b_eff.
    This lets the attention matmul consume `values` in natural [k, d]
    layout (k on partitions) — no transpose of the big tensor anywhere.
  - Softmax normalization deferred: G~ = E @ values accumulated
    unnormalized in PSUM; Z = column sums of E obtained with a ones-matmul
    (output replicated across all 128 partitions so the normalizing
    multiply needs no partition broadcast).

Sharding: data-parallel over batch, B=32 over 8 cores -> 4 batches/core.
No collectives needed; outputs concatenated on host.
"""

import numpy as np
import ml_dtypes

B, CH, CW, H, FH, FW = 32, 16, 16, 8, 4, 4
NQ = 64
NCELL = CH * CW          # 256 coarse cells (c)
F = FH * FW              # 16 fine positions per cell
NK = NCELL * F           # 4096
D_IN, D_V, D_MODEL = 512, 64, 512
N_CORES = 8
B_LOC = B // N_CORES     # 4
N_KT = 32                # k-tiles of 128: kt = half*16 + f, partition = c_loc
N_DT = 4                 # d_in tiles of 128
HQ = H * NQ              # 512

BF16 = ml_dtypes.bfloat16


def _k_perm():
    """perm[k'] -> original k, where k' = (half*16+f)*128 + c_loc.

    Original key order is (ch, fh, cw, fw):  k = ch*256 + fh*64 + cw*4 + fw.
    New order groups a k-tile as (fixed f=(fh,fw), c = half*128 + c_loc).
    """
    perm = np.zeros(NK, np.int64)
    c = np.arange(NCELL)
    ch_i, cw_i = c // CW, c % CW
    for half in range(2):
        for f in range(F):
            kt = half * F + f
            fh, fw = f // FW, f % FW
            cc = half * 128 + np.arange(128)
            perm[kt * 128:(kt + 1) * 128] = (
                ch_i[cc] * (FH * CW * FW) + fh * (CW * FW) + cw_i[cc] * FW + fw
            )
    return perm


_PERM = _k_perm()
_NC_CACHE = {}


def _build_nc():
    from contextlib import ExitStack

    import concourse.bass as bass
    import concourse.tile as tile
    from concourse import bacc, mybir

    f32 = mybir.dt.float32
    bf16 = mybir.dt.bfloat16

    nc = bacc.Bacc("TRN2", target_bir_lowering=False, debug=False,
                   num_devices=N_CORES)

    values_r = nc.dram_tensor("values_r", [B_LOC, NK, D_IN], bf16,
                              kind="ExternalInput")
    att3_t = nc.dram_tensor("att3_t", [B_LOC, NCELL, HQ], bf16,
                            kind="ExternalInput")
    att12_pair = nc.dram_tensor("att12_pair", [B_LOC, NCELL, F * H * 2], bf16,
                                kind="ExternalInput")
    m_all = nc.dram_tensor("m_all", [128, N_DT * H * D_MODEL], bf16,
                           kind="ExternalInput")
    beff = nc.dram_tensor("beff", [1, D_MODEL], bf16, kind="ExternalInput")
    out = nc.dram_tensor("out", [B_LOC * NQ, D_MODEL], f32,
                         kind="ExternalOutput")

    with tile.TileContext(nc) as tc, ExitStack() as ctx:
        const_pool = ctx.enter_context(tc.tile_pool(name="const", bufs=1))
        a3_pool = ctx.enter_context(tc.tile_pool(name="a3", bufs=2))
        a12r_pool = ctx.enter_context(tc.tile_pool(name="a12r", bufs=2))
        vt_pool = ctx.enter_context(tc.tile_pool(name="vt", bufs=20))
        sc_pool = ctx.enter_context(tc.tile_pool(name="sc", bufs=4))
        et_pool = ctx.enter_context(tc.tile_pool(name="et", bufs=4))
        esum_pool = ctx.enter_context(tc.tile_pool(name="esum", bufs=2))
        zb_pool = ctx.enter_context(tc.tile_pool(name="zb", bufs=2))
        g_pool = ctx.enter_context(tc.tile_pool(name="gps", bufs=1, space="PSUM"))
        z_pool = ctx.enter_context(tc.tile_pool(name="zps", bufs=1, space="PSUM"))
        o_pool = ctx.enter_context(tc.tile_pool(name="ops", bufs=1, space="PSUM"))
        o_sb_pool = ctx.enter_context(tc.tile_pool(name="osb", bufs=2))

        ones_sb = const_pool.tile([128, 128], bf16)
        nc.vector.memset(ones_sb[:], 1.0)
        warm_sb = const_pool.tile([128, D_MODEL], bf16, name="warm_sb")
        nc.vector.memset(warm_sb[:], 1.0)
        warm = o_pool.tile([128, D_MODEL], f32, tag="o", name="warm")
        for wi in range(12):
            nc.tensor.matmul(warm[:], ones_sb[:], warm_sb[:],
                             start=True, stop=True)
        beff_sb = const_pool.tile([1, D_MODEL], bf16)
        nc.sync.dma_start(beff_sb[:], beff.ap())
        # g_all[d_loc, (dt, h, b, q)] : normalized attention output, bf16
        g_all = const_pool.tile([128, N_DT * H * B_LOC * NQ], bf16)

        Q2 = NQ // 2

        def emit_group(b, half, gi, FQ, f0, a3_t, a12r_t):
            """One score group: broadcast multiply + exp for FQ f-positions."""
            a3b = a3_t[half][:]
            in0 = bass.AP(a3b.tensor, a3b.offset,
                          [a3b.ap[0], [0, FQ], [NQ, H], [2, Q2], [1, 2]])
            sc = sc_pool.tile([128, 4 * HQ], bf16, tag="sc",
                              name=f"sc_{b}_{half}_{gi}")
            scb = sc[:]
            out_ap = bass.AP(scb.tensor, scb.offset,
                             [scb.ap[0], [HQ, FQ], [NQ, H], [2, Q2], [1, 2]])
            a12b = a12r_t[half][:]
            in1 = bass.AP(a12b.tensor, a12b.offset + f0 * H * 2,
                          [a12b.ap[0], [H * 2, FQ], [2, H], [0, Q2], [1, 2]])
            nc.vector.tensor_mul(out_ap, in0, in1)
            et = et_pool.tile([128, 4 * HQ], bf16, tag="et",
                              name=f"et_{b}_{half}_{gi}")
            nc.scalar.activation(et[:, :FQ * HQ], sc[:, :FQ * HQ],
                                 mybir.ActivationFunctionType.Exp)
            return et

        def prologue(b):
            """Input DMAs + first score group for batch b — emitted ahead of
            the previous batch's epilogue so the DVE/ACT pipeline stays
            primed across the batch transition."""
            a3_t = [a3_pool.tile([128, HQ], bf16, tag=f"a3_{hf}",
                                 name=f"a3_{b}_{hf}") for hf in range(2)]
            for hf in range(2):
                nc.sync.dma_start(a3_t[hf][:],
                                  att3_t.ap()[b, hf * 128:(hf + 1) * 128, :])
            a12r_t = []
            for hf in range(2):
                a12r = a12r_pool.tile([128, F * H * 2], bf16, tag=f"a12r_{hf}",
                                      name=f"a12r_{b}_{hf}")
                nc.sync.dma_start(a12r[:],
                                  att12_pair.ap()[b, hf * 128:(hf + 1) * 128, :])
                a12r_t.append(a12r)
            groups = [1, 1, 2, 4, 4, 4] if b == 0 else [4, 4, 4, 4]
            et0 = emit_group(b, 0, 0, groups[0], 0, a3_t, a12r_t)
            return a3_t, a12r_t, groups, et0

        pro = prologue(0)
        for b in range(B_LOC):
            a3_t, a12r_t, groups0, et0 = pro
            gps = [g_pool.tile([128, HQ], f32, tag=f"g{dt}", name=f"g_{b}_{dt}",
                               bufs=(2 if dt < 2 else 1))
                   for dt in range(N_DT)]
            esum = esum_pool.tile([128, HQ], bf16)

            for half in range(2):
                groups = groups0 if half == 0 else [4, 4, 4, 4]
                f0 = 0
                for gi, FQ in enumerate(groups):
                    if half == 0 and gi == 0:
                        et = et0
                    else:
                        et = emit_group(b, half, gi, FQ, f0, a3_t, a12r_t)
                    for j in range(FQ):
                        kt = half * F + f0 + j
                        vt = vt_pool.tile([128, D_IN], bf16, tag="vt",
                                          name=f"vt_{b}_{kt}")
                        nc.sync.dma_start(
                            vt[:], values_r.ap()[b, kt * 128:(kt + 1) * 128, :])
                        ets = et[:, j * HQ:(j + 1) * HQ]
                        DEFER = 5
                        if kt < DEFER:
                            if kt == 0:
                                deferred = []
                            for dt in range(2):
                                nc.tensor.matmul(gps[dt][:],
                                                 vt[:, dt * 128:(dt + 1) * 128],
                                                 ets, start=(kt == 0),
                                                 stop=False)
                            deferred.append((vt, ets, kt == 0))
                            if kt == DEFER - 1:
                                for dvt, dets, dstart in deferred:
                                    for dt in range(2, N_DT):
                                        nc.tensor.matmul(
                                            gps[dt][:],
                                            dvt[:, dt * 128:(dt + 1) * 128],
                                            dets, start=dstart, stop=False)
                        else:
                            for dt in range(N_DT):
                                nc.tensor.matmul(gps[dt][:],
                                                 vt[:, dt * 128:(dt + 1) * 128],
                                                 ets,
                                                 start=False,
                                                 stop=(kt == N_KT - 1))
                        if kt == 0:
                            nc.vector.tensor_copy(esum[:], ets)
                        else:
                            nc.vector.tensor_add(esum[:], esum[:], ets)
                    f0 += FQ
                if b == 1 and half == 0:
                    # weights for the output projection, emitted mid-stream so
                    # the transfer never contends with critical prefetches
                    m_sb = const_pool.tile([128, N_DT * H * D_MODEL], bf16,
                                           name="m_sb")
                    mq = N_DT * H * D_MODEL // 4
                    for mi in range(4):
                        nc.sync.dma_start(m_sb[:, mi * mq:(mi + 1) * mq],
                                          m_all.ap()[:, mi * mq:(mi + 1) * mq])

            if b + 1 < B_LOC:
                pro = prologue(b + 1)

            # Z replicated on all partitions via ones-matmul, then 1/Z
            zps = z_pool.tile([128, HQ], f32, tag="z")
            nc.tensor.matmul(zps[:], ones_sb[:], esum[:], start=True, stop=True)
            zb = zb_pool.tile([128, HQ], f32)
            nc.vector.reciprocal_approx_fast(zb[:], zps[:])

            ga_v = g_all[:].rearrange("p (dt h bb q) -> p dt h bb q",
                                      dt=N_DT, h=H, bb=B_LOC)
            for dt in range(N_DT):
                nc.vector.tensor_mul(
                    ga_v[:, dt, :, b, :],
                    gps[dt][:].rearrange("p (h q) -> p h q", h=H),
                    zb[:].rearrange("p (h q) -> p h q", h=H),
                )

            # Output projection for each completed pair of batches
            if b % 2 == 1:
                bq = b // 2
                ops = o_pool.tile([128, D_MODEL], f32, tag="o")
                i = 0
                for dt in range(N_DT):
                    for h in range(H):
                        col = dt * (H * B_LOC * NQ) + h * (B_LOC * NQ) + bq * 128
                        nc.tensor.matmul(
                            ops[:],
                            g_all[:, col:col + 128],
                            m_sb[:, (dt * H + h) * D_MODEL:
                                 (dt * H + h + 1) * D_MODEL],
                            start=(i == 0), stop=False)
                        i += 1
                # bias via K=1 matmul (broadcasts b_eff to all partitions)
                nc.tensor.matmul(ops[:], ones_sb[0:1, :], beff_sb[:],
                                 start=False, stop=True)
                out_sb = o_sb_pool.tile([128, D_MODEL], f32, tag="osb",
                                        name=f"osb_{bq}")
                nc.vector.tensor_copy(out_sb[:], ops[:])
                nc.sync.dma_start(out.ap()[bq * 128:(bq + 1) * 128, :],
                                  out_sb[:])

    nc.compile()
    return nc


def _get_nc():
    if "nc" not in _NC_CACHE:
        _NC_CACHE["nc"] = _build_nc()
    return _NC_CACHE["nc"]


def _host_prep(att12, att3, values, W_v, b_v, W_o, b_o):
    att12 = np.asarray(att12, np.float32)
    att3 = np.asarray(att3, np.float32)
    values = np.asarray(values, np.float32)
    W_v = np.asarray(W_v, np.float32)
    b_v = np.asarray(b_v, np.float32)
    W_o = np.asarray(W_o, np.float32)
    b_o = np.asarray(b_o, np.float32)

    values_r = np.ascontiguousarray(values[:, _PERM, :]).astype(BF16)
    att3_t = np.ascontiguousarray(
        att3.transpose(0, 3, 1, 2).reshape(B, NCELL, HQ)).astype(BF16)
    att12_r = np.ascontiguousarray(
        att12.transpose(0, 1, 2, 4, 5, 3).reshape(B, NCELL, F * H)).astype(BF16)
    att12_pair = np.ascontiguousarray(np.broadcast_to(
        att12_r[:, :, :, None], (B, NCELL, F * H, 2)).reshape(
        B, NCELL, F * H * 2))

    # Per-head folded projection M_h = W_o_h @ W_v_h  [D_MODEL, D_IN]
    Wv3 = W_v.reshape(H, D_V, D_IN)
    Wo3 = W_o.reshape(D_MODEL, H, D_V)
    M = np.einsum("dhv,hvi->hdi", Wo3, Wv3)          # [H, DM, DIN]
    # m_all[d_loc, (dt, h, dm)] = M[h].T[dt*128+d_loc, dm]
    Mt = M.transpose(0, 2, 1)                        # [H, DIN, DM]
    m_all = np.ascontiguousarray(
        Mt.reshape(H, N_DT, 128, D_MODEL).transpose(2, 1, 0, 3)
        .reshape(128, N_DT * H * D_MODEL)).astype(BF16)

    b_eff = b_o + np.einsum("dhv,hv->d", Wo3, b_v.reshape(H, D_V))
    beff = b_eff.reshape(1, D_MODEL).astype(BF16)
    return values_r, att3_t, att12_pair, m_all, beff


def kernel(att12, att3, values, W_v, b_v, W_o, b_o):
    from concourse.bass_utils import run_bass_kernel_spmd

    values_r, att3_t, att12_pair, m_all, beff = _host_prep(
        att12, att3, values, W_v, b_v, W_o, b_o)

    in_maps = []
    for core in range(N_CORES):
        s = slice(core * B_LOC, (core + 1) * B_LOC)
        in_maps.append({
            "values_r": np.ascontiguousarray(values_r[s]),
            "att3_t": np.ascontiguousarray(att3_t[s]),
            "att12_pair": np.ascontiguousarray(att12_pair[s]),
            "m_all": m_all,
            "beff": beff,
        })

    nc = _get_nc()
    res = run_bass_kernel_spmd(nc, in_maps, core_ids=list(range(N_CORES)))
    out = np.concatenate(
        [res.results[i]["out"].reshape(B_LOC, NQ, D_MODEL)
         for i in range(N_CORES)], axis=0)
    return out.astype(np.float32)
